# revision 17
# baseline (speedup 1.0000x reference)
"""Trainium2 Bass kernel for ModalitySpecificLocalSelfAttention (7x7 local window).

Strategy (8 NeuronCores, spatial-parallel over H):
  - Each core owns a 16-row stripe of the 128x128 image; k/v paths get a
    3-row halo (22 rows total). 1x1 convs run as PE matmuls with BN scale
    folded into the weights on the host.
  - Local attention per 8x16 pixel block: one matmul Q_blockT @ K_window
    where K_window is a strided 3D slice of the padded k tensor (no gather),
    giving S [128pix, 308] in PSUM. Softmax: ACT exp -> one DVE
    tensor_tensor_reduce (multiplicative window mask + Z reduction with
    oob-row compensation as the init scalar) -> reciprocal -> in-place
    normalize.
  - A^T and V^T via DMA XBAR transposes split across the two HWDGE queues
    (sync + scalar); V transposes run during the conv phase. AV is 3
    accumulating [128]-contraction matmuls into the same PSUM bank as S.
  - q2's conv epilogue writes straight into block-major layout (no q gather).
  - Final conv: W_a @ attn + W_x @ x accumulated in PSUM + bias.
"""

import sys

for _p in ("/opt/trn_rl_repo", "/root/.axon_site/_ro/trn_rl_repo"):
    if _p not in sys.path:
        sys.path.append(_p)

import ml_dtypes
import numpy as np

import concourse.bass as bass
from concourse import mybir
from concourse.bass_utils import run_bass_kernel_spmd

F32 = mybir.dt.float32
F32R = mybir.dt.float32r
BF16 = mybir.dt.bfloat16

C = 128
H = 128
W = 128
NCORES = 8
RPC = H // NCORES          # 16 rows per core
PAD = 3
HALO = RPC + 2 * PAD       # 22 rows incl halo
WP = W + 2 * PAD           # 134 padded width
BR, BC = 8, 16             # pixel block 8 rows x 16 cols = 128 pixels
NBR, NBC = RPC // BR, W // BC
NBLK = NBR * NBC           # 16 blocks
NR, NC_ = BR + 2 * PAD, BC + 2 * PAD  # neighborhood 14 x 22
NN = NR * NC_              # 308
NN2 = 384                  # padded to transpose granularity
NPIX = RPC * W             # 2048
NHALO = HALO * W           # 2816

EXP_SHIFT = -16.0
DBG = False
CH = 512                   # conv matmul N-chunk (one PSUM bank of f32)

# at-transpose queue assignment: True -> scalar engine queue
ACT_AT = [b % 2 == 0 for b in range(NBLK)]


def _build_program():
    nc = bass.Bass("TRN2", target_bir_lowering=False, debug=False)

    # ---- DRAM I/O ----
    xs_d = nc.dram_tensor("xs", [C, NHALO], BF16, kind="ExternalInput").ap()
    W_NAMES = ("wq1t", "wq2t", "wk1t", "wk2t", "wvt", "wat", "wxt")
    B_NAMES = ("bq1", "bq2", "bk1", "bk2", "bv", "bo")
    wall_d = nc.dram_tensor("wall", [C, 7 * C], BF16, kind="ExternalInput").ap()
    ball_d = nc.dram_tensor("ball", [C, 8], F32, kind="ExternalInput").ap()
    smask_d = nc.dram_tensor("smask", [C, NBR, NN + 1], BF16,
                             kind="ExternalInput").ap()
    y_d = nc.dram_tensor("y", [C, NPIX], F32, kind="ExternalOutput").ap()
    if DBG:
        dbg_names = {}
        for nm, shp, dt in (("d_qb", [C, NBLK * BR * BC], BF16),
                            ("d_kpad", [C, HALO * WP], BF16),
                            ("d_vpad", [C, HALO * WP], BF16),
                            ("d_am4", [C, NBLK * NN2], BF16),
                            ("d_vt", [C, NBLK * 3 * C], BF16),
                            ("d_at", [C, NBLK * 3 * C], BF16),
                            ("d_attn", [C, NPIX], BF16),
                            ("d_zs", [C, NBLK], F32)):
            dbg_names[nm] = nc.dram_tensor(nm, shp, dt, kind="ExternalOutput").ap()

    # ---- SBUF ----
    sb = lambda name, shape, dt: nc.alloc_sbuf_tensor(name, list(shape), dt).ap()
    xs = sb("xs_sb", [C, NHALO], BF16)
    k1 = sb("k1_sb", [C, NHALO], BF16)
    q1 = sb("q1_sb", [C, NPIX], BF16)
    qb = sb("qb_sb", [C, NBLK, BR * BC], BF16)   # q2 in block-major
    kpad = sb("kpad_sb", [C, HALO, WP], BF16)
    vpad = sb("vpad_sb", [C, HALO, WP], BF16)
    attn = sb("attn_sb", [C, RPC, W], BF16)
    wall = sb("wall_sb", [C, 7 * C], BF16)
    ball = sb("ball_sb", [C, 8], F32)
    w_sb = {n: wall[:, bass.ts(k, C)] for k, n in enumerate(W_NAMES)}
    b_sb = {n: ball[:, k:k + 1] for k, n in enumerate(B_NAMES)}
    oobc = ball[:, 6:8]
    smask = sb("smask_sb", [C, NBR, NN + 1], BF16)
    zeros = sb("zeros_sb", [C, CH], F32)
    eshift = sb("eshift_sb", [C, 1], F32)
    vn4 = sb("vn4_sb", [C, NBLK, NN2], BF16)
    am4 = sb("am4_sb", [C, NBLK, NN2], BF16)
    vt16 = sb("vt16_sb", [C, NBLK, 3, C], BF16)
    at16 = sb("at16_sb", [C, NBLK, 3, C], BF16)
    zs4 = sb("zs4_sb", [C, NBLK], F32)
    rz4 = sb("rz4_sb", [C, NBLK], F32)
    yt = sb("yt_sb", [C, 2, CH], F32)

    # ---- PSUM: 2 tensors x 4 slots = 8 banks ----
    PS = [nc.alloc_psum_tensor(f"ps{i}", [C, 4, CH], F32).ap()
          for i in range(2)]

    # ---- semaphores ----
    sem_names = tuple(
        ["sde", "sdw", "sp", "sa", "sv", "sg", "sdtv", "sdtas", "sdtaa",
         "sdout0", "sdout1"]
        + [f"sdx{j}" for j in range(6)])
    sems = {n: nc.alloc_semaphore(n) for n in sem_names}

    ENGS = ("sync", "pe", "act", "dve", "gp")
    plan = {e: [] for e in ENGS}
    cnt = {n: 0 for n in sem_names}

    def op(eng, fn, sem, inc=1):
        plan[eng].append(("op", fn, sem, inc))
        if sem:
            cnt[sem] += inc
            return cnt[sem]
        return None

    def wait(eng, sem, val):
        if val and val > 0:
            plan[eng].append(("w", sem, val))

    RELU = mybir.ActivationFunctionType.Relu
    IDENT = mybir.ActivationFunctionType.Identity
    EXP = mybir.ActivationFunctionType.Exp
    MUL = mybir.AluOpType.mult
    ADD = mybir.AluOpType.add

    # ---- input DMAs ----
    def dma_in(sem, dst, srcd):
        return op("sync",
                  lambda d=dst, s=srcd: nc.sync.dma_start(out=d, in_=s),
                  sem, 16)

    dma_in("sde", wall, wall_d)
    SDE_ALL = dma_in("sde", ball, ball_d)
    for i in range((NHALO + CH - 1) // CH):
        n = min(CH, NHALO - i * CH)
        dma_in(f"sdx{i}", xs[:, bass.ds(i * CH, n)],
               xs_d[:, bass.ds(i * CH, n)])
    SDW_ALL = dma_in("sdw", smask, smask_d)

    # ---- init memsets (gpsimd, idle at start) ----
    op("gp", lambda: nc.gpsimd.memset(kpad[:, :, 0:PAD], 0.0), "sg")
    op("gp", lambda: nc.gpsimd.memset(kpad[:, :, PAD + W:WP], 0.0), "sg")
    op("gp", lambda: nc.gpsimd.memset(vpad[:, :, 0:PAD], 0.0), "sg")
    op("gp", lambda: nc.gpsimd.memset(vpad[:, :, PAD + W:WP], 0.0), "sg")
    op("gp", lambda: nc.gpsimd.memset(vn4[:, :, NN:NN2], 0.0), "sg")
    MEMSETS = op("gp", lambda: nc.gpsimd.memset(am4[:, :, NN + 1:NN2], 0.0),
                 "sg")
    op("dve", lambda: nc.vector.memset(eshift, EXP_SHIFT), "sv")
    op("dve", lambda: nc.vector.memset(zeros, 0.0), "sv")
    # am4[:, b, NN] <- oobc[br]: stt passthrough column for fused Z init
    wait("dve", "sde", SDE_ALL)
    OOBI = None
    for br in range(NBR):
        OOBI = op("dve",
                  lambda o=am4[:, br * NBC:(br + 1) * NBC, NN],
                         i_=zeros[:, 0:NBC], s_=oobc[:, br:br + 1]:
                      nc.vector.tensor_scalar_add(o, i_, s_),
                  "sv")

    # ---- convs ----
    # epilogues: k1/q1/v -> DVE (bias-add + max0), k2/q2 -> ACT RELU
    conv_list = [
        ("k1", "wk1t", xs, 0, NHALO, k1, "bk1"),
        ("q1", "wq1t", xs, PAD * W, NPIX, q1, "bq1"),
        ("v", "wvt", xs, 0, NHALO, vpad, "bv"),
        ("k2", "wk2t", k1, 0, NHALO, kpad, "bk2"),
        ("q2", "wq2t", q1, 0, NPIX, qb, "bq2"),
    ]
    mm_done, epi_done = {}, {}
    last_slot_epi = {}

    MAX = mybir.AluOpType.max

    def emit_conv_chunk(ci, j, tidx, slot):
        cname, wn, rhs, roff, ntot, dst, bn = conv_list[ci]
        n = min(CH, ntot - j * CH)
        ps = PS[tidx][:, slot, :]
        src_conv = {"k2": 0, "q2": 1}.get(cname)
        if cname in ("k1", "v"):
            wait("pe", f"sdx{j}", 16)
        elif cname == "q1":
            wait("pe", f"sdx{j}", 16)
            wait("pe", f"sdx{j + 1}", 16)
        if src_conv is not None:
            m_ = epi_done.get((src_conv, j))
            if m_:
                wait("pe", m_[0], m_[1])
        m_ = last_slot_epi.get((tidx, slot))
        if m_:
            wait("pe", m_[0], m_[1])
        mm_done[(ci, j)] = op(
            "pe",
            lambda p=ps[:, :n], w_=w_sb[wn],
                   r=rhs[:, bass.ds(roff + j * CH, n)]:
                nc.tensor.matmul(p, w_, r, start=True, stop=True),
            "sp")
        if cname in ("k1", "q1", "v"):
            # DVE epilogue: relu(x + b) = (ps + b) max 0
            wait("dve", "sp", mm_done[(ci, j)])
            if cname == "v":
                wait("dve", "sg", MEMSETS)
                nr = n // W
                r0 = (j * CH) // W
                o_ap = dst[:, r0:r0 + nr, PAD:PAD + W]
                p_ap = ps[:, :n].rearrange("p (r w) -> p r w", w=W)
            else:
                o_ap = dst[:, bass.ds(j * CH, n)]
                p_ap = ps[:, :n]
            val = ("sv", op(
                "dve",
                lambda o=o_ap, p=p_ap, b=b_sb[bn], z=zeros[:, :n]:
                    nc.vector.scalar_tensor_tensor(
                        out=o, in0=p, scalar=b, in1=z, op0=ADD, op1=MAX),
                "sv"))
        else:
            wait("act", "sp", mm_done[(ci, j)])
            if cname == "k2":
                nr = n // W
                r0 = (j * CH) // W
                val = ("sa", op(
                    "act",
                    lambda o=dst[:, r0:r0 + nr, PAD:PAD + W],
                           p=ps[:, :n].rearrange("p (r w) -> p r w", w=W),
                           b=b_sb[bn]:
                        nc.scalar.activation(o, p, RELU, bias=b),
                    "sa"))
            else:
                r0 = (j * CH) // W
                br = r0 // BR
                ro = r0 % BR
                val = ("sa", op(
                    "act",
                    lambda o=qb[:, br * NBC:(br + 1) * NBC,
                                ro * BC:(ro + 4) * BC].rearrange(
                                    "p b (r c) -> p r b c", r=4),
                           p=ps[:, :n].rearrange("p (r b c) -> p r b c",
                                                 r=4, b=NBC),
                           b=b_sb[bn]:
                        nc.scalar.activation(o, p, RELU, bias=b),
                    "sa"))
        epi_done[(ci, j)] = val
        last_slot_epi[(tidx, slot)] = val

    wait("pe", "sde", SDE_ALL)
    wait("act", "sde", SDE_ALL)

    # v early (feeds vn/vt), k-path for block-row 0, q-path, rest
    conv_order = (
        [(0, j) for j in range(4)] + [(2, j) for j in range(4)]
        + [(0, j) for j in range(4, 6)] + [(3, j) for j in range(2)]
        + [(2, j) for j in range(4, 6)] + [(3, j) for j in range(2, 4)]
        + [(1, j) for j in range(4)] + [(4, j) for j in range(2)]
        + [(3, j) for j in range(4, 6)] + [(4, j) for j in range(2, 4)]
    )
    for idx, (ci, j) in enumerate(conv_order):
        tidx = 0 if idx < 16 else 1
        slot = idx % 4 if idx < 16 else (idx - 16) % 4
        emit_conv_chunk(ci, j, tidx, slot)

    # ---- vn staging: evens on DVE, odds on GPS (conv phase) ----
    vn_done = {}
    wait("gp", "sg", MEMSETS)
    for b in range(NBLK):
        br, cb = b // NBC, b % NBC
        need = 3 if br == 0 else 5
        eng = "dve" if b % 2 == 0 else "gp"
        sem = "sv" if b % 2 == 0 else "sg"
        m_ = epi_done[(2, need)]
        wait(eng, m_[0], m_[1])
        cp = nc.vector.tensor_copy if b % 2 == 0 else nc.gpsimd.tensor_copy
        vn_done[b] = (sem, op(
            eng,
            lambda d=vn4[:, b, 0:NN].rearrange("p (r w) -> p r w", r=NR),
                   s_=vpad[:, br * BR:br * BR + NR,
                           cb * BC:cb * BC + NC_], c=cp:
                c(d, s_),
            sem))

    # ---- vt transposes: evens on ACT, odds on SYNC (conv phase) ----
    vt_done = {}
    for b in range(NBLK):
        eng = "act" if b % 2 == 0 else "sync"
        dma = nc.scalar if b % 2 == 0 else nc.sync
        sem = "sdtaa" if b % 2 == 0 else "sdtv"
        wait(eng, *vn_done[b])
        vt_done[b] = (sem, op(
            eng,
            lambda o=vt16[:, b], s_=vn4[:, b, :], d=dma:
                d.dma_start(out=o, in_=s_, transpose=True),
            sem, 16))

    # ---- attention ----
    # PSUM: S rotates banks 0-5 (PS0 slots 0-3, PS1 slots 0-1);
    # AV packs 8 [C,128] regions into PS1 slots 2-3; oconv reuses PS0.
    s_done, exp_done, norm_done, at_done, av_done, ac_done = ({} for _ in
                                                              range(6))

    def s_bank(b):
        return b % 2, (b // 2) % 4

    def av_region(b):
        t, s = s_bank(b)
        return PS[t][:, s, NN2:CH]

    def st_s(b):
        br, cb = b // NBC, b % NBC
        t, s = s_bank(b)
        ps = PS[t][:, s, :]
        wait("pe", *epi_done[(4, 2 * br + 1)])
        wait("pe", *epi_done[(3, 3 if br == 0 else 5)])
        if b >= 8:
            wait("pe", "sa", exp_done[b - 8])
            wait("pe", "sv", ac_done[b - 8])
        else:
            m_ = last_slot_epi.get((t, s))
            if m_:
                wait("pe", m_[0], m_[1])
        s_done[b] = op(
            "pe",
            lambda o=ps[:, 0:NN], l=qb[:, b, :],
                   r=kpad[:, br * BR:br * BR + NR, cb * BC:cb * BC + NC_]:
                nc.tensor.matmul(o, l, r, start=True, stop=True),
            "sp")

    def st_exp(b):
        t, s = s_bank(b)
        ps = PS[t][:, s, :]
        wait("act", "sp", s_done[b])
        exp_done[b] = op(
            "act",
            lambda o=am4[:, b, 0:NN], i_=ps[:, 0:NN]:
                nc.scalar.activation(o, i_, EXP, bias=eshift),
            "sa")

    def st_softmax(b):
        br = b // NBC
        wait("dve", "sa", exp_done[b])
        v1 = op("dve",
           lambda o=am4[:, b, 0:NN + 1], i0=am4[:, b, 0:NN + 1],
                  i1=smask[:, br, :], z=zs4[:, b:b + 1]:
               nc.vector.scalar_tensor_tensor(
                   out=o, in0=i0, scalar=1.0, in1=i1,
                   op0=MUL, op1=MUL, accum_out=z),
           "sv")
        wait("dve", "sv", v1)
        v3 = op("dve",
           lambda o=rz4[:, b:b + 1], i_=zs4[:, b:b + 1]:
               nc.vector.reciprocal(o, i_),
           "sv")
        wait("dve", "sv", v3)
        norm_done[b] = op(
            "dve",
            lambda o=am4[:, b, 0:NN + 1], i_=am4[:, b, 0:NN + 1],
                   s_=rz4[:, b:b + 1]:
                nc.vector.tensor_scalar_mul(o, i_, s_),
            "sv")

    def st_atrans(b):
        eng = "act" if ACT_AT[b] else "sync"
        dma = nc.scalar if ACT_AT[b] else nc.sync
        sem = "sdtaa" if ACT_AT[b] else "sdtas"
        wait(eng, "sv", norm_done[b])
        at_done[b] = (sem, op(
            eng,
            lambda o=at16[:, b], s_=am4[:, b, :], d=dma:
                d.dma_start(out=o, in_=s_, transpose=True),
            sem, 16))

    def st_av(b):
        ps = av_region(b)
        wait("pe", *vt_done[b])
        wait("pe", *at_done[b])
        for ch in range(3):
            av_done[b] = op(
                "pe",
                lambda o=ps, l=vt16[:, b, ch],
                       r=at16[:, b, ch], st=(ch == 0), sp_=(ch == 2):
                    nc.tensor.matmul(o, l, r, start=st, stop=sp_),
                "sp")

    def st_accopy(b):
        br, cb = b // NBC, b % NBC
        ps = av_region(b)
        wait("dve", "sp", av_done[b])
        ac_done[b] = op(
            "dve",
            lambda o=attn[:, br * BR:(br + 1) * BR,
                          cb * BC:(cb + 1) * BC],
                   i_=ps.rearrange("p (r w) -> p r w", w=BC):
                nc.vector.tensor_copy(o, i_),
            "sv")

    # front-load all S matmuls; EXP right behind; softmax on DVE;
    # at on both queues trailing by 4 EXPs; AV/accopy trail
    for b in range(NBLK):
        st_s(b)
        st_exp(b)
        st_softmax(b)
        if b >= 4:
            st_atrans(b - 4)
        if b >= 6:
            st_av(b - 6)
            st_accopy(b - 6)
    for b in range(NBLK - 4, NBLK):
        st_atrans(b)
    for b in range(NBLK - 6, NBLK):
        st_av(b)
        st_accopy(b)

    # ---- output conv (PS0 banks, after S12-15 EXPs) ----
    attn_flat = attn.rearrange("p r w -> p (r w)")
    oc_done, yt_done = {}, {}
    for c in range(NPIX // CH):
        ps = PS[c % 2][:, c // 2, :]
        wait("pe", "sa", exp_done[c + 8])
        wait("pe", "sv", ac_done[max(c + 8, 8 * (c // 2 + 1) - 1)])
        if c >= 2:
            wait("pe", "sv", yt_done[c - 2])
        op("pe",
           lambda o=ps, l=w_sb["wat"], r=attn_flat[:, bass.ts(c, CH)]:
               nc.tensor.matmul(o, l, r, start=True, stop=False),
           "sp")
        oc_done[c] = op(
            "pe",
            lambda o=ps, l=w_sb["wxt"],
                   r=xs[:, bass.ds(PAD * W + c * CH, CH)]:
                nc.tensor.matmul(o, l, r, start=False, stop=True),
            "sp")
        wait("dve", "sp", oc_done[c])
        if c >= 2:
            wait("dve", f"sdout{c % 2}", 16 * (c // 2))
        yt_done[c] = op(
            "dve",
            lambda o=yt[:, c % 2, :], i_=ps, b=b_sb["bo"]:
                nc.vector.tensor_scalar_add(o, i_, b),
            "sv")
        wait("sync", "sv", yt_done[c])
        op("sync",
           lambda o=y_d[:, bass.ts(c, CH)], i_=yt[:, c % 2, :]:
               nc.sync.dma_start(out=o, in_=i_),
           f"sdout{c % 2}", 16)

    if DBG:
        for sname in ("sp", "sa", "sv"):
            wait("sync", sname, cnt[sname])
        for nm, t in (("d_qb", qb), ("d_kpad", kpad), ("d_vpad", vpad),
                      ("d_am4", am4), ("d_vt", vt16), ("d_at", at16),
                      ("d_attn", attn), ("d_zs", zs4)):
            if len(t.shape) == 3:
                flat = t.rearrange("p a b -> p (a b)")
            elif len(t.shape) == 4:
                flat = t.rearrange("p a b c -> p (a b c)")
            else:
                flat = t
            op("sync", lambda o=dbg_names[nm], i_=flat:
               nc.sync.dma_start(out=o, in_=i_), "sdout0", 16)

    # ---- tail barrier ----
    for sname in ("sp", "sa", "sv", "sg", "sdtv", "sdtas", "sdtaa",
                  "sdout0", "sdout1", "sde", "sdw"):
        wait("sync", sname, cnt[sname])
    for j in range(6):
        wait("sync", f"sdx{j}", cnt[f"sdx{j}"])

    # ---- emit ----
    def run(eng_name, eng_obj):
        hwm = {}
        for item in plan[eng_name]:
            if item[0] == "w":
                _, s_, v = item
                if hwm.get(s_, 0) >= v:
                    continue
                hwm[s_] = v
                eng_obj.wait_ge(sems[s_], v)
            else:
                _, fn, s_, inc = item
                inst = fn()
                if s_:
                    inst.then_inc(sems[s_], inc)

    with nc.Block() as block:
        @block.sync
        def _(e):
            run("sync", e)

        @block.tensor
        def _(e):
            run("pe", e)

        @block.scalar
        def _(e):
            run("act", e)

        @block.vector
        def _(e):
            run("dve", e)

        @block.gpsimd
        def _(e):
            run("gp", e)

    with nc.Block() as block2:
        @block2.sync
        def _(e):
            for n in sem_names:
                nc.sync.sem_clear(sems[n])

    return nc


_PROGRAM = None


def _host_inputs(x, w_q1, s_q1, b_q1, w_q2, s_q2, b_q2,
                 w_k1, s_k1, b_k1, w_k2, s_k2, b_k2,
                 w_v, s_v, b_v, w_o, s_o, b_o):
    """Per-core input dicts (numpy) for the SPMD program."""
    def foldT(w, s):
        return np.ascontiguousarray((s[:, None] * w).T.astype(ml_dtypes.bfloat16))

    wq1t, wq2t = foldT(w_q1, s_q1), foldT(w_q2, s_q2)
    wk1t, wk2t = foldT(w_k1, s_k1), foldT(w_k2, s_k2)
    wvt = foldT(w_v, s_v)
    wo = s_o[:, None] * w_o
    wat = np.ascontiguousarray(wo[:, :C].T.astype(ml_dtypes.bfloat16))
    wxt = np.ascontiguousarray(wo[:, C:].T.astype(ml_dtypes.bfloat16))

    col = lambda b: np.ascontiguousarray(b.astype(np.float32)[:, None])

    # window-validity over the 14x22 neighborhood, per block pixel
    valid = np.zeros((BR * BC, NR, NC_), bool)
    for r in range(BR):
        for c in range(BC):
            p = r * BC + c
            valid[p, r:r + 7, c:c + 7] = True

    X = np.asarray(x, np.float32).reshape(C, H, W)
    wall = np.concatenate([wq1t, wq2t, wk1t, wk2t, wvt, wat, wxt], axis=1)
    shared = dict(wall=np.ascontiguousarray(wall))

    e16v = np.float32(np.exp(EXP_SHIFT))
    in_maps = []
    for core in range(NCORES):
        h0 = core * RPC
        xsb = np.zeros((C, HALO, W), np.float32)
        lo, hi = h0 - PAD, h0 + RPC + PAD
        slo, shi = max(lo, 0), min(hi, H)
        xsb[:, slo - lo:shi - lo] = X[:, slo:shi]

        # per-block-row multiplicative 0/1 mask (0 for off-window, OOB-row,
        # and pad cols) and out-of-image-row Z compensation
        smask = np.zeros((NBR, BR * BC, NN + 1), np.float32)
        smask[:, :, NN] = 1.0
        oobc = np.zeros((NBR, BR * BC), np.float32)
        for brr in range(NBR):
            rowok = np.array([0 <= h0 + brr * BR + ri - PAD < H
                              for ri in range(NR)])
            m = (valid & rowok[None, :, None]).astype(np.float32)
            smask[brr, :, :NN] = m.reshape(BR * BC, NN)
            for r in range(BR):
                n_oob = sum(1 for i in range(7)
                            if not (0 <= h0 + brr * BR + r - PAD + i < H))
                oobc[brr, r * BC:(r + 1) * BC] = 7 * n_oob * e16v
        m = dict(shared)
        m["xs"] = np.ascontiguousarray(
            xsb.reshape(C, NHALO).astype(ml_dtypes.bfloat16))
        m["smask"] = np.ascontiguousarray(
            smask.transpose(1, 0, 2).astype(ml_dtypes.bfloat16))
        m["ball"] = np.ascontiguousarray(np.concatenate(
            [col(b_q1), col(b_q2), col(b_k1), col(b_k2), col(b_v),
             col(b_o), oobc.T.astype(np.float32)], axis=1))
        in_maps.append(m)
    return in_maps


def kernel(**inputs):
    global _PROGRAM
    if _PROGRAM is None:
        _PROGRAM = _build_program()
    in_maps = _host_inputs(**{k: np.asarray(v) for k, v in inputs.items()})
    res = run_bass_kernel_spmd(_PROGRAM, in_maps, core_ids=list(range(NCORES)))
    stripes = [np.asarray(r["y"]).reshape(C, RPC, W) for r in res.results]
    return np.concatenate(stripes, axis=1).reshape(1, C, H, W)


if __name__ == "__main__":
    rng = np.random.default_rng(0)
    fake = {"x": rng.standard_normal((1, C, H, W), np.float32)}
    for n in ("q1", "q2", "k1", "k2", "v", "o"):
        cin = 2 * C if n == "o" else C
        fake["w_" + n] = rng.standard_normal((C, cin), np.float32) / np.sqrt(cin)
        fake["s_" + n] = rng.uniform(0.5, 1.5, C).astype(np.float32)
        fake["b_" + n] = (rng.standard_normal(C) * 0.1).astype(np.float32)
    out = kernel(**fake)
    print("kernel output", out.shape, out.dtype)


# revision 19
# speedup vs baseline: 1.0072x; 1.0072x over previous
"""Trainium2 Bass kernel for ModalitySpecificLocalSelfAttention (7x7 local window).

Strategy (8 NeuronCores, spatial-parallel over H):
  - Each core owns a 16-row stripe of the 128x128 image; k/v paths get a
    3-row halo (22 rows total). 1x1 convs run as PE matmuls with BN scale
    folded into the weights on the host.
  - Local attention per 8x16 pixel block: one matmul Q_blockT @ K_window
    where K_window is a strided 3D slice of the padded k tensor (no gather),
    giving S [128pix, 308] in PSUM. Softmax: ACT exp -> one DVE
    tensor_tensor_reduce (multiplicative window mask + Z reduction with
    oob-row compensation as the init scalar) -> reciprocal -> in-place
    normalize.
  - A^T and V^T via DMA XBAR transposes split across the two HWDGE queues
    (sync + scalar); V transposes run during the conv phase. AV is 3
    accumulating [128]-contraction matmuls into the same PSUM bank as S.
  - q2's conv epilogue writes straight into block-major layout (no q gather).
  - Final conv: W_a @ attn + W_x @ x accumulated in PSUM + bias.
"""

import sys

for _p in ("/opt/trn_rl_repo", "/root/.axon_site/_ro/trn_rl_repo"):
    if _p not in sys.path:
        sys.path.append(_p)

import ml_dtypes
import numpy as np

import concourse.bass as bass
from concourse import mybir
from concourse.bass_utils import run_bass_kernel_spmd

F32 = mybir.dt.float32
F32R = mybir.dt.float32r
BF16 = mybir.dt.bfloat16

C = 128
H = 128
W = 128
NCORES = 8
RPC = H // NCORES          # 16 rows per core
PAD = 3
HALO = RPC + 2 * PAD       # 22 rows incl halo
WP = W + 2 * PAD           # 134 padded width
BR, BC = 8, 16             # pixel block 8 rows x 16 cols = 128 pixels
NBR, NBC = RPC // BR, W // BC
NBLK = NBR * NBC           # 16 blocks
NR, NC_ = BR + 2 * PAD, BC + 2 * PAD  # neighborhood 14 x 22
NN = NR * NC_              # 308
NN2 = 384                  # padded to transpose granularity
NPIX = RPC * W             # 2048
NHALO = HALO * W           # 2816

EXP_SHIFT = -16.0
DBG = False
CH = 512                   # conv matmul N-chunk (one PSUM bank of f32)

# at-transpose queue assignment: True -> scalar engine queue
ACT_AT = [b % 2 == 0 for b in range(NBLK)]


def _build_program():
    nc = bass.Bass("TRN2", target_bir_lowering=False, debug=False)

    # ---- DRAM I/O ----
    xs_d = nc.dram_tensor("xs", [C, NHALO], BF16, kind="ExternalInput").ap()
    W_NAMES = ("wq1t", "wq2t", "wk1t", "wk2t", "wvt", "wat", "wxt")
    B_NAMES = ("bq1", "bq2", "bk1", "bk2", "bv", "bo")
    wall_d = nc.dram_tensor("wall", [C, 7 * C], BF16, kind="ExternalInput").ap()
    ball_d = nc.dram_tensor("ball", [C, 8], F32, kind="ExternalInput").ap()
    smask_d = nc.dram_tensor("smask", [C, NBR, NN + 1], BF16,
                             kind="ExternalInput").ap()
    y_d = nc.dram_tensor("y", [C, NPIX], F32, kind="ExternalOutput").ap()
    if DBG:
        dbg_names = {}
        for nm, shp, dt in (("d_qb", [C, NBLK * BR * BC], BF16),
                            ("d_kpad", [C, HALO * WP], BF16),
                            ("d_vpad", [C, HALO * WP], BF16),
                            ("d_am4", [C, NBLK * NN2], BF16),
                            ("d_vt", [C, NBLK * 3 * C], BF16),
                            ("d_at", [C, NBLK * 3 * C], BF16),
                            ("d_attn", [C, NPIX], BF16),
                            ("d_zs", [C, NBLK], F32)):
            dbg_names[nm] = nc.dram_tensor(nm, shp, dt, kind="ExternalOutput").ap()

    # ---- SBUF ----
    sb = lambda name, shape, dt: nc.alloc_sbuf_tensor(name, list(shape), dt).ap()
    xs = sb("xs_sb", [C, NHALO], BF16)
    k1 = sb("k1_sb", [C, NHALO], BF16)
    q1 = sb("q1_sb", [C, NPIX], BF16)
    qb = sb("qb_sb", [C, NBLK, BR * BC], BF16)   # q2 in block-major
    kpad = sb("kpad_sb", [C, HALO, WP], BF16)
    vpad = sb("vpad_sb", [C, HALO, WP], BF16)
    attn = sb("attn_sb", [C, RPC, W], BF16)
    wall = sb("wall_sb", [C, 7 * C], BF16)
    ball = sb("ball_sb", [C, 8], F32)
    w_sb = {n: wall[:, bass.ts(k, C)] for k, n in enumerate(W_NAMES)}
    b_sb = {n: ball[:, k:k + 1] for k, n in enumerate(B_NAMES)}
    oobc = ball[:, 6:8]
    smask = sb("smask_sb", [C, NBR, NN + 1], BF16)
    zeros = sb("zeros_sb", [C, CH], F32)
    eshift = sb("eshift_sb", [C, 1], F32)
    vn4 = sb("vn4_sb", [C, NBLK, NN2], BF16)
    am4 = sb("am4_sb", [C, NBLK, NN2], BF16)
    vt16 = sb("vt16_sb", [C, NBLK, 3, C], BF16)
    at16 = sb("at16_sb", [C, NBLK, 3, C], BF16)
    zs4 = sb("zs4_sb", [C, NBLK], F32)
    rz4 = sb("rz4_sb", [C, NBLK], F32)
    yt = sb("yt_sb", [C, 2, CH], F32)

    # ---- PSUM: 2 tensors x 4 slots = 8 banks ----
    PS = [nc.alloc_psum_tensor(f"ps{i}", [C, 4, CH], F32).ap()
          for i in range(2)]

    # ---- semaphores ----
    sem_names = tuple(
        ["sde", "sdw", "sp", "sa", "sv", "sg", "sdtv", "sdtas", "sdtaa",
         "sdout0", "sdout1"]
        + [f"sdx{j}" for j in range(6)])
    sems = {n: nc.alloc_semaphore(n) for n in sem_names}

    ENGS = ("sync", "pe", "act", "dve", "gp")
    plan = {e: [] for e in ENGS}
    cnt = {n: 0 for n in sem_names}

    def op(eng, fn, sem, inc=1):
        plan[eng].append(("op", fn, sem, inc))
        if sem:
            cnt[sem] += inc
            return cnt[sem]
        return None

    def wait(eng, sem, val):
        if val and val > 0:
            plan[eng].append(("w", sem, val))

    RELU = mybir.ActivationFunctionType.Relu
    IDENT = mybir.ActivationFunctionType.Identity
    EXP = mybir.ActivationFunctionType.Exp
    MUL = mybir.AluOpType.mult
    ADD = mybir.AluOpType.add

    # ---- input DMAs ----
    def dma_in(sem, dst, srcd):
        return op("sync",
                  lambda d=dst, s=srcd: nc.sync.dma_start(out=d, in_=s),
                  sem, 16)

    dma_in("sde", wall, wall_d)
    SDE_ALL = dma_in("sde", ball, ball_d)
    for i in range((NHALO + CH - 1) // CH):
        n = min(CH, NHALO - i * CH)
        dma_in(f"sdx{i}", xs[:, bass.ds(i * CH, n)],
               xs_d[:, bass.ds(i * CH, n)])
    SDW_ALL = dma_in("sdw", smask, smask_d)

    # ---- init memsets (gpsimd, idle at start) ----
    op("gp", lambda: nc.gpsimd.memset(kpad[:, :, 0:PAD], 0.0), "sg")
    op("gp", lambda: nc.gpsimd.memset(kpad[:, :, PAD + W:WP], 0.0), "sg")
    op("gp", lambda: nc.gpsimd.memset(vpad[:, :, 0:PAD], 0.0), "sg")
    op("gp", lambda: nc.gpsimd.memset(vpad[:, :, PAD + W:WP], 0.0), "sg")
    op("gp", lambda: nc.gpsimd.memset(vn4[:, :, NN:NN2], 0.0), "sg")
    MEMSETS = op("gp", lambda: nc.gpsimd.memset(am4[:, :, NN + 1:NN2], 0.0),
                 "sg")
    op("dve", lambda: nc.vector.memset(eshift, EXP_SHIFT), "sv")
    op("dve", lambda: nc.vector.memset(zeros, 0.0), "sv")
    # am4[:, b, NN] <- oobc[br]: stt passthrough column for fused Z init
    wait("dve", "sde", SDE_ALL)
    OOBI = None
    for br in range(NBR):
        OOBI = op("dve",
                  lambda o=am4[:, br * NBC:(br + 1) * NBC, NN],
                         i_=zeros[:, 0:NBC], s_=oobc[:, br:br + 1]:
                      nc.vector.tensor_scalar_add(o, i_, s_),
                  "sv")

    # ---- convs ----
    # epilogues: k1/q1/v -> DVE (bias-add + max0), k2/q2 -> ACT RELU
    conv_list = [
        ("k1", "wk1t", xs, 0, NHALO, k1, "bk1"),
        ("q1", "wq1t", xs, PAD * W, NPIX, q1, "bq1"),
        ("v", "wvt", xs, 0, NHALO, vpad, "bv"),
        ("k2", "wk2t", k1, 0, NHALO, kpad, "bk2"),
        ("q2", "wq2t", q1, 0, NPIX, qb, "bq2"),
    ]
    mm_done, epi_done = {}, {}
    last_slot_epi = {}

    MAX = mybir.AluOpType.max

    def emit_conv_chunk(ci, j, tidx, slot):
        cname, wn, rhs, roff, ntot, dst, bn = conv_list[ci]
        n = min(CH, ntot - j * CH)
        ps = PS[tidx][:, slot, :]
        src_conv = {"k2": 0, "q2": 1}.get(cname)
        if cname in ("k1", "v"):
            wait("pe", f"sdx{j}", 16)
        elif cname == "q1":
            wait("pe", f"sdx{j}", 16)
            wait("pe", f"sdx{j + 1}", 16)
        if src_conv is not None:
            m_ = epi_done.get((src_conv, j))
            if m_:
                wait("pe", m_[0], m_[1])
        m_ = last_slot_epi.get((tidx, slot))
        if m_:
            wait("pe", m_[0], m_[1])
        mm_done[(ci, j)] = op(
            "pe",
            lambda p=ps[:, :n], w_=w_sb[wn],
                   r=rhs[:, bass.ds(roff + j * CH, n)]:
                nc.tensor.matmul(p, w_, r, start=True, stop=True),
            "sp")
        if cname in ("k1", "q1", "v", "q2"):
            wait("act", "sp", mm_done[(ci, j)])
            if cname == "v":
                wait("act", "sg", MEMSETS)
                nr = n // W
                r0 = (j * CH) // W
                o_ap = dst[:, r0:r0 + nr, PAD:PAD + W]
                p_ap = ps[:, :n].rearrange("p (r w) -> p r w", w=W)
            elif cname == "q2":
                r0 = (j * CH) // W
                br = r0 // BR
                ro = r0 % BR
                o_ap = qb[:, br * NBC:(br + 1) * NBC,
                          ro * BC:(ro + 4) * BC].rearrange(
                              "p b (r c) -> p r b c", r=4)
                p_ap = ps[:, :n].rearrange("p (r b c) -> p r b c",
                                           r=4, b=NBC)
            else:
                o_ap = dst[:, bass.ds(j * CH, n)]
                p_ap = ps[:, :n]
            val = ("sa", op(
                "act",
                lambda o=o_ap, p=p_ap, b=b_sb[bn]:
                    nc.scalar.activation(o, p, RELU, bias=b),
                "sa"))
        else:  # k2 -> DVE
            # DVE epilogue: relu(x + b) = (ps + b) max 0
            wait("dve", "sp", mm_done[(ci, j)])
            nr = n // W
            r0 = (j * CH) // W
            o_ap = dst[:, r0:r0 + nr, PAD:PAD + W]
            p_ap = ps[:, :n].rearrange("p (r w) -> p r w", w=W)
            val = ("sv", op(
                "dve",
                lambda o=o_ap, p=p_ap, b=b_sb[bn], z=zeros[:, :n]:
                    nc.vector.scalar_tensor_tensor(
                        out=o, in0=p, scalar=b, in1=z, op0=ADD, op1=MAX),
                "sv"))
        epi_done[(ci, j)] = val
        last_slot_epi[(tidx, slot)] = val

    wait("pe", "sde", SDE_ALL)
    wait("act", "sde", SDE_ALL)

    # v early (feeds vn/vt), k-path for block-row 0, q-path, rest
    conv_order = (
        [(0, j) for j in range(4)] + [(2, j) for j in range(4)]
        + [(0, j) for j in range(4, 6)] + [(3, j) for j in range(2)]
        + [(2, j) for j in range(4, 6)] + [(3, j) for j in range(2, 4)]
        + [(1, j) for j in range(4)] + [(4, j) for j in range(2)]
        + [(3, j) for j in range(4, 6)] + [(4, j) for j in range(2, 4)]
    )
    for idx, (ci, j) in enumerate(conv_order):
        tidx = 0 if idx < 16 else 1
        slot = idx % 4 if idx < 16 else (idx - 16) % 4
        emit_conv_chunk(ci, j, tidx, slot)

    # ---- vn staging: evens on DVE, odds on GPS (conv phase) ----
    vn_done = {}
    wait("gp", "sg", MEMSETS)
    for b in range(NBLK):
        br, cb = b // NBC, b % NBC
        need = 3 if br == 0 else 5
        eng = "dve" if b % 2 == 0 else "gp"
        sem = "sv" if b % 2 == 0 else "sg"
        m_ = epi_done[(2, need)]
        wait(eng, m_[0], m_[1])
        cp = nc.vector.tensor_copy if b % 2 == 0 else nc.gpsimd.tensor_copy
        vn_done[b] = (sem, op(
            eng,
            lambda d=vn4[:, b, 0:NN].rearrange("p (r w) -> p r w", r=NR),
                   s_=vpad[:, br * BR:br * BR + NR,
                           cb * BC:cb * BC + NC_], c=cp:
                c(d, s_),
            sem))

    # ---- vt transposes: evens on ACT, odds on SYNC (conv phase) ----
    vt_done = {}
    for b in range(NBLK):
        eng = "sync"
        dma = nc.sync
        sem = "sdtv"
        wait(eng, *vn_done[b])
        vt_done[b] = (sem, op(
            eng,
            lambda o=vt16[:, b], s_=vn4[:, b, :], d=dma:
                d.dma_start(out=o, in_=s_, transpose=True),
            sem, 16))

    # ---- attention ----
    # PSUM: S rotates banks 0-5 (PS0 slots 0-3, PS1 slots 0-1);
    # AV packs 8 [C,128] regions into PS1 slots 2-3; oconv reuses PS0.
    s_done, exp_done, norm_done, at_done, av_done, ac_done = ({} for _ in
                                                              range(6))

    def s_bank(b):
        return b % 2, (b // 2) % 4

    def av_region(b):
        t, s = s_bank(b)
        return PS[t][:, s, NN2:CH]

    def st_s(b):
        br, cb = b // NBC, b % NBC
        t, s = s_bank(b)
        ps = PS[t][:, s, :]
        wait("pe", *epi_done[(4, 2 * br + 1)])
        wait("pe", *epi_done[(3, 3 if br == 0 else 5)])
        if b >= 8:
            wait("pe", "sa", exp_done[b - 8])
            wait("pe", "sv", ac_done[b - 8])
        else:
            m_ = last_slot_epi.get((t, s))
            if m_:
                wait("pe", m_[0], m_[1])
        s_done[b] = op(
            "pe",
            lambda o=ps[:, 0:NN], l=qb[:, b, :],
                   r=kpad[:, br * BR:br * BR + NR, cb * BC:cb * BC + NC_]:
                nc.tensor.matmul(o, l, r, start=True, stop=True),
            "sp")

    def st_exp(b):
        t, s = s_bank(b)
        ps = PS[t][:, s, :]
        wait("act", "sp", s_done[b])
        exp_done[b] = op(
            "act",
            lambda o=am4[:, b, 0:NN], i_=ps[:, 0:NN]:
                nc.scalar.activation(o, i_, EXP, bias=eshift),
            "sa")

    def st_softmax(b):
        br = b // NBC
        wait("dve", "sa", exp_done[b])
        v1 = op("dve",
           lambda o=am4[:, b, 0:NN + 1], i0=am4[:, b, 0:NN + 1],
                  i1=smask[:, br, :], z=zs4[:, b:b + 1]:
               nc.vector.scalar_tensor_tensor(
                   out=o, in0=i0, scalar=1.0, in1=i1,
                   op0=MUL, op1=MUL, accum_out=z),
           "sv")
        wait("dve", "sv", v1)
        v3 = op("dve",
           lambda o=rz4[:, b:b + 1], i_=zs4[:, b:b + 1]:
               nc.vector.reciprocal(o, i_),
           "sv")
        wait("dve", "sv", v3)
        norm_done[b] = op(
            "dve",
            lambda o=am4[:, b, 0:NN + 1], i_=am4[:, b, 0:NN + 1],
                   s_=rz4[:, b:b + 1]:
                nc.vector.tensor_scalar_mul(o, i_, s_),
            "sv")

    def st_atrans(b):
        eng = "act" if ACT_AT[b] else "sync"
        dma = nc.scalar if ACT_AT[b] else nc.sync
        sem = "sdtaa" if ACT_AT[b] else "sdtas"
        wait(eng, "sv", norm_done[b])
        at_done[b] = (sem, op(
            eng,
            lambda o=at16[:, b], s_=am4[:, b, :], d=dma:
                d.dma_start(out=o, in_=s_, transpose=True),
            sem, 16))

    def st_av(b):
        ps = av_region(b)
        wait("pe", *vt_done[b])
        wait("pe", *at_done[b])
        for ch in range(3):
            av_done[b] = op(
                "pe",
                lambda o=ps, l=vt16[:, b, ch],
                       r=at16[:, b, ch], st=(ch == 0), sp_=(ch == 2):
                    nc.tensor.matmul(o, l, r, start=st, stop=sp_),
                "sp")

    def st_accopy(b):
        br, cb = b // NBC, b % NBC
        ps = av_region(b)
        wait("dve", "sp", av_done[b])
        ac_done[b] = op(
            "dve",
            lambda o=attn[:, br * BR:(br + 1) * BR,
                          cb * BC:(cb + 1) * BC],
                   i_=ps.rearrange("p (r w) -> p r w", w=BC):
                nc.vector.tensor_copy(o, i_),
            "sv")

    # front-load all S matmuls; EXP right behind; softmax on DVE;
    # at on both queues trailing by 4 EXPs; AV/accopy trail
    for b in range(NBLK):
        st_s(b)
        st_exp(b)
        st_softmax(b)
        if b >= 4:
            st_atrans(b - 4)
        if b >= 6:
            st_av(b - 6)
            st_accopy(b - 6)
    for b in range(NBLK - 4, NBLK):
        st_atrans(b)
    for b in range(NBLK - 6, NBLK):
        st_av(b)
        st_accopy(b)

    # ---- output conv (PS0 banks, after S12-15 EXPs) ----
    attn_flat = attn.rearrange("p r w -> p (r w)")
    oc_done, yt_done = {}, {}
    for c in range(NPIX // CH):
        ps = PS[c % 2][:, c // 2, :]
        wait("pe", "sa", exp_done[c + 8])
        wait("pe", "sv", ac_done[max(c + 8, 8 * (c // 2 + 1) - 1)])
        if c >= 2:
            wait("pe", "sv", yt_done[c - 2])
        op("pe",
           lambda o=ps, l=w_sb["wat"], r=attn_flat[:, bass.ts(c, CH)]:
               nc.tensor.matmul(o, l, r, start=True, stop=False),
           "sp")
        oc_done[c] = op(
            "pe",
            lambda o=ps, l=w_sb["wxt"],
                   r=xs[:, bass.ds(PAD * W + c * CH, CH)]:
                nc.tensor.matmul(o, l, r, start=False, stop=True),
            "sp")
        wait("dve", "sp", oc_done[c])
        if c >= 2:
            wait("dve", f"sdout{c % 2}", 16 * (c // 2))
        yt_done[c] = op(
            "dve",
            lambda o=yt[:, c % 2, :], i_=ps, b=b_sb["bo"]:
                nc.vector.tensor_scalar_add(o, i_, b),
            "sv")
        wait("sync", "sv", yt_done[c])
        op("sync",
           lambda o=y_d[:, bass.ts(c, CH)], i_=yt[:, c % 2, :]:
               nc.sync.dma_start(out=o, in_=i_),
           f"sdout{c % 2}", 16)

    if DBG:
        for sname in ("sp", "sa", "sv"):
            wait("sync", sname, cnt[sname])
        for nm, t in (("d_qb", qb), ("d_kpad", kpad), ("d_vpad", vpad),
                      ("d_am4", am4), ("d_vt", vt16), ("d_at", at16),
                      ("d_attn", attn), ("d_zs", zs4)):
            if len(t.shape) == 3:
                flat = t.rearrange("p a b -> p (a b)")
            elif len(t.shape) == 4:
                flat = t.rearrange("p a b c -> p (a b c)")
            else:
                flat = t
            op("sync", lambda o=dbg_names[nm], i_=flat:
               nc.sync.dma_start(out=o, in_=i_), "sdout0", 16)

    # ---- tail barrier ----
    for sname in ("sp", "sa", "sv", "sg", "sdtv", "sdtas", "sdtaa",
                  "sdout0", "sdout1", "sde", "sdw"):
        wait("sync", sname, cnt[sname])
    for j in range(6):
        wait("sync", f"sdx{j}", cnt[f"sdx{j}"])

    # ---- emit ----
    def run(eng_name, eng_obj):
        hwm = {}
        for item in plan[eng_name]:
            if item[0] == "w":
                _, s_, v = item
                if hwm.get(s_, 0) >= v:
                    continue
                hwm[s_] = v
                eng_obj.wait_ge(sems[s_], v)
            else:
                _, fn, s_, inc = item
                inst = fn()
                if s_:
                    inst.then_inc(sems[s_], inc)

    with nc.Block() as block:
        @block.sync
        def _(e):
            run("sync", e)

        @block.tensor
        def _(e):
            run("pe", e)

        @block.scalar
        def _(e):
            run("act", e)

        @block.vector
        def _(e):
            run("dve", e)

        @block.gpsimd
        def _(e):
            run("gp", e)

    with nc.Block() as block2:
        @block2.sync
        def _(e):
            for n in sem_names:
                nc.sync.sem_clear(sems[n])

    return nc


_PROGRAM = None


def _host_inputs(x, w_q1, s_q1, b_q1, w_q2, s_q2, b_q2,
                 w_k1, s_k1, b_k1, w_k2, s_k2, b_k2,
                 w_v, s_v, b_v, w_o, s_o, b_o):
    """Per-core input dicts (numpy) for the SPMD program."""
    def foldT(w, s):
        return np.ascontiguousarray((s[:, None] * w).T.astype(ml_dtypes.bfloat16))

    wq1t, wq2t = foldT(w_q1, s_q1), foldT(w_q2, s_q2)
    wk1t, wk2t = foldT(w_k1, s_k1), foldT(w_k2, s_k2)
    wvt = foldT(w_v, s_v)
    wo = s_o[:, None] * w_o
    wat = np.ascontiguousarray(wo[:, :C].T.astype(ml_dtypes.bfloat16))
    wxt = np.ascontiguousarray(wo[:, C:].T.astype(ml_dtypes.bfloat16))

    col = lambda b: np.ascontiguousarray(b.astype(np.float32)[:, None])

    # window-validity over the 14x22 neighborhood, per block pixel
    valid = np.zeros((BR * BC, NR, NC_), bool)
    for r in range(BR):
        for c in range(BC):
            p = r * BC + c
            valid[p, r:r + 7, c:c + 7] = True

    X = np.asarray(x, np.float32).reshape(C, H, W)
    wall = np.concatenate([wq1t, wq2t, wk1t, wk2t, wvt, wat, wxt], axis=1)
    shared = dict(wall=np.ascontiguousarray(wall))

    e16v = np.float32(np.exp(EXP_SHIFT))
    in_maps = []
    for core in range(NCORES):
        h0 = core * RPC
        xsb = np.zeros((C, HALO, W), np.float32)
        lo, hi = h0 - PAD, h0 + RPC + PAD
        slo, shi = max(lo, 0), min(hi, H)
        xsb[:, slo - lo:shi - lo] = X[:, slo:shi]

        # per-block-row multiplicative 0/1 mask (0 for off-window, OOB-row,
        # and pad cols) and out-of-image-row Z compensation
        smask = np.zeros((NBR, BR * BC, NN + 1), np.float32)
        smask[:, :, NN] = 1.0
        oobc = np.zeros((NBR, BR * BC), np.float32)
        for brr in range(NBR):
            rowok = np.array([0 <= h0 + brr * BR + ri - PAD < H
                              for ri in range(NR)])
            m = (valid & rowok[None, :, None]).astype(np.float32)
            smask[brr, :, :NN] = m.reshape(BR * BC, NN)
            for r in range(BR):
                n_oob = sum(1 for i in range(7)
                            if not (0 <= h0 + brr * BR + r - PAD + i < H))
                oobc[brr, r * BC:(r + 1) * BC] = 7 * n_oob * e16v
        m = dict(shared)
        m["xs"] = np.ascontiguousarray(
            xsb.reshape(C, NHALO).astype(ml_dtypes.bfloat16))
        m["smask"] = np.ascontiguousarray(
            smask.transpose(1, 0, 2).astype(ml_dtypes.bfloat16))
        m["ball"] = np.ascontiguousarray(np.concatenate(
            [col(b_q1), col(b_q2), col(b_k1), col(b_k2), col(b_v),
             col(b_o), oobc.T.astype(np.float32)], axis=1))
        in_maps.append(m)
    return in_maps


def kernel(**inputs):
    global _PROGRAM
    if _PROGRAM is None:
        _PROGRAM = _build_program()
    in_maps = _host_inputs(**{k: np.asarray(v) for k, v in inputs.items()})
    res = run_bass_kernel_spmd(_PROGRAM, in_maps, core_ids=list(range(NCORES)))
    stripes = [np.asarray(r["y"]).reshape(C, RPC, W) for r in res.results]
    return np.concatenate(stripes, axis=1).reshape(1, C, H, W)


if __name__ == "__main__":
    rng = np.random.default_rng(0)
    fake = {"x": rng.standard_normal((1, C, H, W), np.float32)}
    for n in ("q1", "q2", "k1", "k2", "v", "o"):
        cin = 2 * C if n == "o" else C
        fake["w_" + n] = rng.standard_normal((C, cin), np.float32) / np.sqrt(cin)
        fake["s_" + n] = rng.uniform(0.5, 1.5, C).astype(np.float32)
        fake["b_" + n] = (rng.standard_normal(C) * 0.1).astype(np.float32)
    out = kernel(**fake)
    print("kernel output", out.shape, out.dtype)
